# revision 18
# baseline (speedup 1.0000x reference)
"""Trainium2 Bass kernel for nn_BitwiseHashing.

Computes out = tanh(mean_l(x) @ W.T + b) for x:[12,8192,1024] f32,
W:[64,1024], b:[64] -> out:[8192,64].

Strategy (data-parallel over 8 NeuronCores):
  - shard x along batch dim: 1024 rows per core (48 MiB each, streamed).
  - host pre-transposes W to wt = (W.T / L) [1024,64]; bias shipped as [1,64].
  - per 128-row block: stream 12 L-slices (contiguous 512 KiB DMAs),
    accumulate with DVE adds, PE-transpose the 8 [128,128] d-chunks of the
    sum, matmul against wt chunks accumulating in PSUM [128,64] (bias
    pre-loaded via a C=1 ones-matmul), tanh on ScalarE, DMA out [128,64].
"""

import numpy as np

import concourse.bacc as bacc
import concourse.mybir as mybir
from concourse import tile
from concourse.masks import make_identity
from concourse.bass_utils import run_bass_kernel_spmd

L, B, D, K = 12, 8192, 1024, 64
NCORES = 8
BS = B // NCORES      # 1024 batch rows per core
P = 128               # partitions
NBLK = BS // P        # 8 row blocks per core
NDC = D // P          # 8 contraction chunks
F32 = mybir.dt.float32

_nc_cache = None


def _build():
    global _nc_cache
    if _nc_cache is not None:
        return _nc_cache

    nc = bacc.Bacc("TRN2", target_bir_lowering=False, debug=False)
    x = nc.dram_tensor("x", [L, BS, D], F32, kind="ExternalInput")
    wt = nc.dram_tensor("wt", [D, K], F32, kind="ExternalInput")
    bias = nc.dram_tensor("bias", [1, K], F32, kind="ExternalInput")
    y = nc.dram_tensor("y", [BS, K], F32, kind="ExternalOutput")

    with tile.TileContext(nc) as tc:
        with (
            tc.tile_pool(name="const", bufs=1) as cpool,
            tc.tile_pool(name="xin", bufs=26) as xpool,
            tc.tile_pool(name="xt", bufs=2) as tpool,
            tc.tile_pool(name="out", bufs=3) as opool,
            tc.tile_pool(name="pt", bufs=2, space="PSUM") as pt_pool,
            tc.tile_pool(name="po", bufs=2, space="PSUM") as po_pool,
        ):
            # constants go over the SWDGE queue to keep both HWDGE rings
            # free for the x stream from t=0
            wt_sb = cpool.tile([P, NDC * K], F32)
            for dc in range(NDC):
                nc.gpsimd.dma_start(
                    out=wt_sb[:, dc * K:(dc + 1) * K],
                    in_=wt.ap()[dc * P:(dc + 1) * P, :],
                )
            bias_sb = cpool.tile([1, K], F32)
            nc.gpsimd.dma_start(out=bias_sb[:], in_=bias.ap())
            ones_sb = cpool.tile([1, P], F32)
            nc.gpsimd.memset(ones_sb[:], 1.0)
            ident = cpool.tile([P, P], F32)
            make_identity(nc, ident[:])

            xap = x.ap()
            yap = y.ap()

            def issue_loads(blk):
                b0 = blk * P
                xt = []
                for l in range(L):
                    xl = xpool.tile([P, D], F32)
                    eng = nc.sync if l % 2 == 0 else nc.scalar
                    eng.dma_start(out=xl[:], in_=xap[l, b0:b0 + P, :])
                    xt.append(xl)
                return xt

            def reduce(xt):
                # two independent running chains, one per DMA ring: the
                # even tiles (sync ring) and odd tiles (scalar ring) each
                # complete in FIFO order within their ring, so each chain
                # only ever waits on its own ring and inter-ring skew
                # cannot stall the reduction
                accE, accO = xt[0], xt[1]
                for l in range(2, L, 2):
                    nc.vector.tensor_add(
                        out=accE[:], in0=accE[:], in1=xt[l][:]
                    )
                    nc.vector.tensor_add(
                        out=accO[:], in0=accO[:], in1=xt[l + 1][:]
                    )
                return accE, accO

            def project(accE, accO):
                # Combine + transpose + copy + matmul as a wavefront over
                # the two PSUM-bank-aligned halves of D: half 0 flows
                # through PE/ACT while half 1 is still combining on DVE.
                H = D // 2
                pt_all = pt_pool.tile([P, D], F32)
                xt_all = tpool.tile([P, D], F32)
                po = po_pool.tile([P, K], F32)
                # bias broadcast across partitions: ones[1,128].T @ bias[1,64]
                nc.tensor.matmul(
                    po[:], lhsT=ones_sb[:], rhs=bias_sb[:], start=True, stop=False
                )
                for h in range(2):
                    h0 = h * H
                    nc.vector.tensor_add(
                        out=accE[:, h0:h0 + H],
                        in0=accE[:, h0:h0 + H],
                        in1=accO[:, h0:h0 + H],
                    )
                    for dc in range(h * NDC // 2, (h + 1) * NDC // 2):
                        nc.tensor.transpose(
                            pt_all[:, dc * P:(dc + 1) * P],
                            accE[:, dc * P:(dc + 1) * P],
                            ident[:],
                        )
                    nc.scalar.copy(
                        out=xt_all[:, h0:h0 + H], in_=pt_all[:, h0:h0 + H]
                    )
                    for dc in range(h * NDC // 2, (h + 1) * NDC // 2):
                        nc.tensor.matmul(
                            po[:],
                            lhsT=xt_all[:, dc * P:(dc + 1) * P],
                            rhs=wt_sb[:, dc * K:(dc + 1) * K],
                            start=False,
                            stop=(dc == NDC - 1),
                        )
                return po

            def finish(blk, po):
                b0 = blk * P
                ot = opool.tile([P, K], F32)
                nc.scalar.activation(
                    ot[:], po[:], mybir.ActivationFunctionType.Tanh
                )
                nc.sync.dma_start(out=yap[b0:b0 + P, :], in_=ot[:])

            # Emission order per block: adds(n) -> loads(n+1) -> psum/matmul
            # stage(n) -> tanh+y(n-1). This keeps every ACT/sync DMA trigger
            # for block n+1 AHEAD of block n's copy/tanh/y in the engine
            # FIFOs, so the two x-stream rings never stall behind compute.
            xt = issue_loads(0)
            prev_po = None
            for blk in range(NBLK):
                accE, accO = reduce(xt)
                if blk + 1 < NBLK:
                    xt = issue_loads(blk + 1)
                po = project(accE, accO)
                if prev_po is not None:
                    finish(blk - 1, prev_po)
                prev_po = po
            finish(NBLK - 1, prev_po)

    nc.compile()
    _nc_cache = nc
    return nc


def _ensure_ntff_hook():
    """Register the axon NTFF profile hook if the image's antenv lacks it."""
    import sys
    import types

    try:
        from antenv.axon_hooks import get_axon_ntff_profile_hook  # noqa: F401
        return
    except ImportError:
        pass
    import antenv

    mod = types.ModuleType("antenv.axon_hooks")
    mod._hook = None

    def set_axon_ntff_profile_hook(h):
        mod._hook = h

    def get_axon_ntff_profile_hook():
        return mod._hook

    mod.set_axon_ntff_profile_hook = set_axon_ntff_profile_hook
    mod.get_axon_ntff_profile_hook = get_axon_ntff_profile_hook
    sys.modules["antenv.axon_hooks"] = mod
    antenv.axon_hooks = mod
    try:
        from trn_agent_boot.trn_boot import _ntff_profile_via_ctypes

        mod._hook = _ntff_profile_via_ctypes("/opt/axon/libaxon_pjrt.so")
    except Exception:
        mod._hook = None


def _run(inputs, trace=False, **kwargs):
    x = np.asarray(inputs["x"], dtype=np.float32)
    W = np.asarray(inputs["W"], dtype=np.float32)
    b = np.asarray(inputs["b"], dtype=np.float32)
    wt = np.ascontiguousarray(W.T).astype(np.float32) * np.float32(1.0 / L)
    bias = np.ascontiguousarray(b.reshape(1, K)).astype(np.float32)
    in_maps = [
        {
            "x": np.ascontiguousarray(x[:, c * BS:(c + 1) * BS, :]),
            "wt": wt,
            "bias": bias,
        }
        for c in range(NCORES)
    ]
    if trace:
        _ensure_ntff_hook()
        import concourse.bass_utils as bu

        bu.upload_artifacts = lambda tmpdir: "local://skipped"
    nc = _build()
    res = run_bass_kernel_spmd(
        nc, in_maps, core_ids=list(range(NCORES)), trace=trace, **kwargs
    )
    y = np.concatenate([r["y"] for r in res.results], axis=0)
    return y, res


def kernel(**inputs):
    y, _ = _run(inputs)
    return y


# revision 20
# speedup vs baseline: 1.1296x; 1.1296x over previous
"""Trainium2 Bass kernel for nn_BitwiseHashing.

Computes out = tanh(mean_l(x) @ W.T + b) for x:[12,8192,1024] f32,
W:[64,1024], b:[64] -> out:[8192,64].

Strategy (data-parallel over 8 NeuronCores):
  - shard x along batch dim: 1024 rows per core (48 MiB each, streamed).
  - host pre-transposes W to wt = (W.T / L) [1024,64]; bias shipped as [1,64].
  - per 128-row block: stream 12 L-slices (contiguous 512 KiB DMAs),
    accumulate with DVE adds, PE-transpose the 8 [128,128] d-chunks of the
    sum, matmul against wt chunks accumulating in PSUM [128,64] (bias
    pre-loaded via a C=1 ones-matmul), tanh on ScalarE, DMA out [128,64].
"""

import numpy as np

import concourse.bacc as bacc
import concourse.mybir as mybir
from concourse import tile
from concourse.masks import make_identity
from concourse.bass_utils import run_bass_kernel_spmd

L, B, D, K = 12, 8192, 1024, 64
NCORES = 8
BS = B // NCORES      # 1024 batch rows per core
P = 128               # partitions
NBLK = BS // P        # 8 row blocks per core
NDC = D // P          # 8 contraction chunks
F32 = mybir.dt.float32

_nc_cache = None


def _build():
    global _nc_cache
    if _nc_cache is not None:
        return _nc_cache

    nc = bacc.Bacc("TRN2", target_bir_lowering=False, debug=False)
    x = nc.dram_tensor("x", [L, BS, D], F32, kind="ExternalInput")
    wt = nc.dram_tensor("wt", [D, K], F32, kind="ExternalInput")
    bias = nc.dram_tensor("bias", [1, K], F32, kind="ExternalInput")
    y = nc.dram_tensor("y", [BS, K], F32, kind="ExternalOutput")

    with tile.TileContext(nc) as tc:
        with (
            tc.tile_pool(name="const", bufs=1) as cpool,
            tc.tile_pool(name="xin", bufs=26) as xpool,
            tc.tile_pool(name="xt", bufs=2) as tpool,
            tc.tile_pool(name="out", bufs=3) as opool,
            tc.tile_pool(name="pt", bufs=2, space="PSUM") as pt_pool,
            tc.tile_pool(name="po", bufs=2, space="PSUM") as po_pool,
        ):
            # constants go over the SWDGE queue to keep both HWDGE rings
            # free for the x stream from t=0
            wt_sb = cpool.tile([P, NDC * K], F32)
            for dc in range(NDC):
                nc.gpsimd.dma_start(
                    out=wt_sb[:, dc * K:(dc + 1) * K],
                    in_=wt.ap()[dc * P:(dc + 1) * P, :],
                )
            bias_sb = cpool.tile([1, K], F32)
            nc.gpsimd.dma_start(out=bias_sb[:], in_=bias.ap())
            ones_sb = cpool.tile([1, P], F32)
            nc.gpsimd.memset(ones_sb[:], 1.0)
            ident = cpool.tile([P, P], F32)
            make_identity(nc, ident[:])

            xap = x.ap()
            yap = y.ap()

            def issue_loads(blk):
                b0 = blk * P
                xt = []
                for l in range(L):
                    xl = xpool.tile([P, D], F32)
                    eng = nc.sync if l % 2 == 0 else nc.scalar
                    eng.dma_start(out=xl[:], in_=xap[l, b0:b0 + P, :])
                    xt.append(xl)
                return xt

            def reduce(xt):
                # two independent running chains, one per DMA ring: the
                # even tiles (sync ring) and odd tiles (scalar ring) each
                # complete in FIFO order within their ring, so each chain
                # only ever waits on its own ring and inter-ring skew
                # cannot stall the reduction
                accE, accO = xt[0], xt[1]
                for l in range(2, L, 2):
                    nc.vector.tensor_add(
                        out=accE[:], in0=accE[:], in1=xt[l][:]
                    )
                    nc.vector.tensor_add(
                        out=accO[:], in0=accO[:], in1=xt[l + 1][:]
                    )
                # final combine split into the two PSUM-bank-aligned halves
                # of D so the projection can wavefront: half 0 transposes
                # while half 1 is still combining
                H = D // 2
                nc.vector.tensor_add(
                    out=accE[:, 0:H], in0=accE[:, 0:H], in1=accO[:, 0:H]
                )
                nc.vector.tensor_add(
                    out=accE[:, H:D], in0=accE[:, H:D], in1=accO[:, H:D]
                )
                return accE

            def project(acc):
                H = D // 2
                pt_all = pt_pool.tile([P, D], F32)
                xt_all = tpool.tile([P, D], F32)
                po = po_pool.tile([P, K], F32)
                # bias broadcast across partitions: ones[1,128].T @ bias[1,64]
                nc.tensor.matmul(
                    po[:], lhsT=ones_sb[:], rhs=bias_sb[:], start=True, stop=False
                )
                for h in range(2):
                    h0 = h * H
                    for dc in range(h * NDC // 2, (h + 1) * NDC // 2):
                        nc.tensor.transpose(
                            pt_all[:, dc * P:(dc + 1) * P],
                            acc[:, dc * P:(dc + 1) * P],
                            ident[:],
                        )
                    nc.scalar.copy(
                        out=xt_all[:, h0:h0 + H], in_=pt_all[:, h0:h0 + H]
                    )
                    for dc in range(h * NDC // 2, (h + 1) * NDC // 2):
                        nc.tensor.matmul(
                            po[:],
                            lhsT=xt_all[:, dc * P:(dc + 1) * P],
                            rhs=wt_sb[:, dc * K:(dc + 1) * K],
                            start=False,
                            stop=(dc == NDC - 1),
                        )
                return po

            def finish(blk, po):
                b0 = blk * P
                ot = opool.tile([P, K], F32)
                nc.scalar.activation(
                    ot[:], po[:], mybir.ActivationFunctionType.Tanh
                )
                nc.sync.dma_start(out=yap[b0:b0 + P, :], in_=ot[:])

            # Emission order per block: adds(n) -> loads(n+1) -> psum/matmul
            # stage(n) -> tanh+y(n-1). This keeps every ACT/sync DMA trigger
            # for block n+1 AHEAD of block n's copy/tanh/y in the engine
            # FIFOs, so the two x-stream rings never stall behind compute.
            xt = issue_loads(0)
            prev_po = None
            for blk in range(NBLK):
                acc = reduce(xt)
                if blk + 1 < NBLK:
                    xt = issue_loads(blk + 1)
                po = project(acc)
                if prev_po is not None:
                    finish(blk - 1, prev_po)
                prev_po = po
            finish(NBLK - 1, prev_po)

    nc.compile()
    _nc_cache = nc
    return nc


def _ensure_ntff_hook():
    """Register the axon NTFF profile hook if the image's antenv lacks it."""
    import sys
    import types

    try:
        from antenv.axon_hooks import get_axon_ntff_profile_hook  # noqa: F401
        return
    except ImportError:
        pass
    import antenv

    mod = types.ModuleType("antenv.axon_hooks")
    mod._hook = None

    def set_axon_ntff_profile_hook(h):
        mod._hook = h

    def get_axon_ntff_profile_hook():
        return mod._hook

    mod.set_axon_ntff_profile_hook = set_axon_ntff_profile_hook
    mod.get_axon_ntff_profile_hook = get_axon_ntff_profile_hook
    sys.modules["antenv.axon_hooks"] = mod
    antenv.axon_hooks = mod
    try:
        from trn_agent_boot.trn_boot import _ntff_profile_via_ctypes

        mod._hook = _ntff_profile_via_ctypes("/opt/axon/libaxon_pjrt.so")
    except Exception:
        mod._hook = None


def _run(inputs, trace=False, **kwargs):
    x = np.asarray(inputs["x"], dtype=np.float32)
    W = np.asarray(inputs["W"], dtype=np.float32)
    b = np.asarray(inputs["b"], dtype=np.float32)
    wt = np.ascontiguousarray(W.T).astype(np.float32) * np.float32(1.0 / L)
    bias = np.ascontiguousarray(b.reshape(1, K)).astype(np.float32)
    in_maps = [
        {
            "x": np.ascontiguousarray(x[:, c * BS:(c + 1) * BS, :]),
            "wt": wt,
            "bias": bias,
        }
        for c in range(NCORES)
    ]
    if trace:
        _ensure_ntff_hook()
        import concourse.bass_utils as bu

        bu.upload_artifacts = lambda tmpdir: "local://skipped"
    nc = _build()
    res = run_bass_kernel_spmd(
        nc, in_maps, core_ids=list(range(NCORES)), trace=trace, **kwargs
    )
    y = np.concatenate([r["y"] for r in res.results], axis=0)
    return y, res


def kernel(**inputs):
    y, _ = _run(inputs)
    return y


# revision 21
# speedup vs baseline: 1.1504x; 1.0184x over previous
"""Trainium2 Bass kernel for nn_BitwiseHashing.

Computes out = tanh(mean_l(x) @ W.T + b) for x:[12,8192,1024] f32,
W:[64,1024], b:[64] -> out:[8192,64].

Strategy (data-parallel over 8 NeuronCores):
  - shard x along batch dim: 1024 rows per core (48 MiB each, streamed).
  - host pre-transposes W to wt = (W.T / L) [1024,64]; bias shipped as [1,64].
  - per 128-row block: stream 12 L-slices (contiguous 512 KiB DMAs),
    accumulate with DVE adds, PE-transpose the 8 [128,128] d-chunks of the
    sum, matmul against wt chunks accumulating in PSUM [128,64] (bias
    pre-loaded via a C=1 ones-matmul), tanh on ScalarE, DMA out [128,64].
"""

import numpy as np

import concourse.bacc as bacc
import concourse.mybir as mybir
from concourse import tile
from concourse.masks import make_identity
from concourse.bass_utils import run_bass_kernel_spmd

L, B, D, K = 12, 8192, 1024, 64
NCORES = 8
BS = B // NCORES      # 1024 batch rows per core
P = 128               # partitions
NBLK = BS // P        # 8 row blocks per core
NDC = D // P          # 8 contraction chunks
F32 = mybir.dt.float32

_nc_cache = None


def _build():
    global _nc_cache
    if _nc_cache is not None:
        return _nc_cache

    nc = bacc.Bacc("TRN2", target_bir_lowering=False, debug=False)
    x = nc.dram_tensor("x", [L, BS, D], F32, kind="ExternalInput")
    wt = nc.dram_tensor("wt", [D, K], F32, kind="ExternalInput")
    bias = nc.dram_tensor("bias", [1, K], F32, kind="ExternalInput")
    y = nc.dram_tensor("y", [BS, K], F32, kind="ExternalOutput")

    with tile.TileContext(nc) as tc:
        with (
            tc.tile_pool(name="const", bufs=1) as cpool,
            tc.tile_pool(name="xin", bufs=26) as xpool,
            tc.tile_pool(name="xt", bufs=2) as tpool,
            tc.tile_pool(name="out", bufs=3) as opool,
            tc.tile_pool(name="pt", bufs=2, space="PSUM") as pt_pool,
            tc.tile_pool(name="po", bufs=2, space="PSUM") as po_pool,
        ):
            # constants go over the SWDGE queue to keep both HWDGE rings
            # free for the x stream from t=0
            wt_sb = cpool.tile([P, NDC * K], F32)
            for dc in range(NDC):
                nc.gpsimd.dma_start(
                    out=wt_sb[:, dc * K:(dc + 1) * K],
                    in_=wt.ap()[dc * P:(dc + 1) * P, :],
                )
            bias_sb = cpool.tile([1, K], F32)
            nc.gpsimd.dma_start(out=bias_sb[:], in_=bias.ap())
            ones_sb = cpool.tile([1, P], F32)
            nc.gpsimd.memset(ones_sb[:], 1.0)
            ident = cpool.tile([P, P], F32)
            make_identity(nc, ident[:])

            xap = x.ap()
            yap = y.ap()

            def issue_loads(blk):
                b0 = blk * P
                xt = []
                for l in range(L):
                    xl = xpool.tile([P, D], F32)
                    eng = nc.sync if l % 2 == 0 else nc.scalar
                    eng.dma_start(out=xl[:], in_=xap[l, b0:b0 + P, :])
                    xt.append(xl)
                return xt

            def reduce(xt):
                # two independent running chains, one per DMA ring: the
                # even tiles (sync ring) and odd tiles (scalar ring) each
                # complete in FIFO order within their ring, so each chain
                # only ever waits on its own ring and inter-ring skew
                # cannot stall the reduction
                accE, accO = xt[0], xt[1]
                for l in range(2, L, 2):
                    nc.vector.tensor_add(
                        out=accE[:], in0=accE[:], in1=xt[l][:]
                    )
                    nc.vector.tensor_add(
                        out=accO[:], in0=accO[:], in1=xt[l + 1][:]
                    )
                nc.vector.tensor_add(out=accE[:], in0=accE[:], in1=accO[:])
                return accE

            def project(acc):
                H = D // 2
                pt_all = pt_pool.tile([P, D], F32)
                xt_all = tpool.tile([P, D], F32)
                po = po_pool.tile([P, K], F32)
                # bias broadcast across partitions: ones[1,128].T @ bias[1,64]
                nc.tensor.matmul(
                    po[:], lhsT=ones_sb[:], rhs=bias_sb[:], start=True, stop=False
                )
                for h in range(2):
                    h0 = h * H
                    for dc in range(h * NDC // 2, (h + 1) * NDC // 2):
                        nc.tensor.transpose(
                            pt_all[:, dc * P:(dc + 1) * P],
                            acc[:, dc * P:(dc + 1) * P],
                            ident[:],
                        )
                    nc.scalar.copy(
                        out=xt_all[:, h0:h0 + H], in_=pt_all[:, h0:h0 + H]
                    )
                    for dc in range(h * NDC // 2, (h + 1) * NDC // 2):
                        nc.tensor.matmul(
                            po[:],
                            lhsT=xt_all[:, dc * P:(dc + 1) * P],
                            rhs=wt_sb[:, dc * K:(dc + 1) * K],
                            start=False,
                            stop=(dc == NDC - 1),
                        )
                return po

            def finish(blk, po):
                b0 = blk * P
                ot = opool.tile([P, K], F32)
                nc.scalar.activation(
                    ot[:], po[:], mybir.ActivationFunctionType.Tanh
                )
                nc.sync.dma_start(out=yap[b0:b0 + P, :], in_=ot[:])

            # Emission order per block: adds(n) -> loads(n+1) -> psum/matmul
            # stage(n) -> tanh+y(n-1). This keeps every ACT/sync DMA trigger
            # for block n+1 AHEAD of block n's copy/tanh/y in the engine
            # FIFOs, so the two x-stream rings never stall behind compute.
            xt = issue_loads(0)
            prev_po = None
            for blk in range(NBLK):
                acc = reduce(xt)
                if blk + 1 < NBLK:
                    xt = issue_loads(blk + 1)
                po = project(acc)
                if prev_po is not None:
                    finish(blk - 1, prev_po)
                prev_po = po
            finish(NBLK - 1, prev_po)

    nc.compile()
    _nc_cache = nc
    return nc


def _ensure_ntff_hook():
    """Register the axon NTFF profile hook if the image's antenv lacks it."""
    import sys
    import types

    try:
        from antenv.axon_hooks import get_axon_ntff_profile_hook  # noqa: F401
        return
    except ImportError:
        pass
    import antenv

    mod = types.ModuleType("antenv.axon_hooks")
    mod._hook = None

    def set_axon_ntff_profile_hook(h):
        mod._hook = h

    def get_axon_ntff_profile_hook():
        return mod._hook

    mod.set_axon_ntff_profile_hook = set_axon_ntff_profile_hook
    mod.get_axon_ntff_profile_hook = get_axon_ntff_profile_hook
    sys.modules["antenv.axon_hooks"] = mod
    antenv.axon_hooks = mod
    try:
        from trn_agent_boot.trn_boot import _ntff_profile_via_ctypes

        mod._hook = _ntff_profile_via_ctypes("/opt/axon/libaxon_pjrt.so")
    except Exception:
        mod._hook = None


def _run(inputs, trace=False, **kwargs):
    x = np.asarray(inputs["x"], dtype=np.float32)
    W = np.asarray(inputs["W"], dtype=np.float32)
    b = np.asarray(inputs["b"], dtype=np.float32)
    wt = np.ascontiguousarray(W.T).astype(np.float32) * np.float32(1.0 / L)
    bias = np.ascontiguousarray(b.reshape(1, K)).astype(np.float32)
    in_maps = [
        {
            "x": np.ascontiguousarray(x[:, c * BS:(c + 1) * BS, :]),
            "wt": wt,
            "bias": bias,
        }
        for c in range(NCORES)
    ]
    if trace:
        _ensure_ntff_hook()
        import concourse.bass_utils as bu

        bu.upload_artifacts = lambda tmpdir: "local://skipped"
    nc = _build()
    res = run_bass_kernel_spmd(
        nc, in_maps, core_ids=list(range(NCORES)), trace=trace, **kwargs
    )
    y = np.concatenate([r["y"] for r in res.results], axis=0)
    return y, res


def kernel(**inputs):
    y, _ = _run(inputs)
    return y


# revision 22
# speedup vs baseline: 1.1584x; 1.0069x over previous
"""Trainium2 Bass kernel for nn_BitwiseHashing.

Computes out = tanh(mean_l(x) @ W.T + b) for x:[12,8192,1024] f32,
W:[64,1024], b:[64] -> out:[8192,64].

Strategy (data-parallel over 8 NeuronCores):
  - shard x along batch dim: 1024 rows per core (48 MiB each, streamed).
  - host pre-transposes W to wt = (W.T / L) [1024,64]; bias shipped as [1,64].
  - per 128-row block: stream 12 L-slices (contiguous 512 KiB DMAs),
    accumulate with DVE adds, PE-transpose the 8 [128,128] d-chunks of the
    sum, matmul against wt chunks accumulating in PSUM [128,64] (bias
    pre-loaded via a C=1 ones-matmul), tanh on ScalarE, DMA out [128,64].
"""

import numpy as np

import concourse.bacc as bacc
import concourse.mybir as mybir
from concourse import tile
from concourse.masks import make_identity
from concourse.bass_utils import run_bass_kernel_spmd

L, B, D, K = 12, 8192, 1024, 64
NCORES = 8
BS = B // NCORES      # 1024 batch rows per core
P = 128               # partitions
NBLK = BS // P        # 8 row blocks per core
NDC = D // P          # 8 contraction chunks
F32 = mybir.dt.float32

_nc_cache = None


def _build():
    global _nc_cache
    if _nc_cache is not None:
        return _nc_cache

    nc = bacc.Bacc("TRN2", target_bir_lowering=False, debug=False)
    x = nc.dram_tensor("x", [L, BS, D], F32, kind="ExternalInput")
    wt = nc.dram_tensor("wt", [D, K], F32, kind="ExternalInput")
    bias = nc.dram_tensor("bias", [1, K], F32, kind="ExternalInput")
    y = nc.dram_tensor("y", [BS, K], F32, kind="ExternalOutput")

    with tile.TileContext(nc) as tc:
        with (
            tc.tile_pool(name="const", bufs=1) as cpool,
            tc.tile_pool(name="xin", bufs=26) as xpool,
            tc.tile_pool(name="xt", bufs=2) as tpool,
            tc.tile_pool(name="out", bufs=3) as opool,
            tc.tile_pool(name="pt", bufs=2, space="PSUM") as pt_pool,
            tc.tile_pool(name="po", bufs=2, space="PSUM") as po_pool,
        ):
            # constants go over the SWDGE queue to keep both HWDGE rings
            # free for the x stream from t=0
            wt_sb = cpool.tile([P, NDC * K], F32)
            for dc in range(NDC):
                nc.gpsimd.dma_start(
                    out=wt_sb[:, dc * K:(dc + 1) * K],
                    in_=wt.ap()[dc * P:(dc + 1) * P, :],
                )
            bias_sb = cpool.tile([1, K], F32)
            nc.gpsimd.dma_start(out=bias_sb[:], in_=bias.ap())
            ones_sb = cpool.tile([1, P], F32)
            nc.gpsimd.memset(ones_sb[:], 1.0)
            ident = cpool.tile([P, P], F32)
            make_identity(nc, ident[:])

            xap = x.ap()
            yap = y.ap()

            def issue_loads(blk):
                b0 = blk * P
                xt = []
                for l in range(L):
                    xl = xpool.tile([P, D], F32)
                    eng = nc.sync if l % 2 == 0 else nc.scalar
                    eng.dma_start(out=xl[:], in_=xap[l, b0:b0 + P, :])
                    xt.append(xl)
                return xt

            def reduce(xt):
                # two independent running chains, one per DMA ring: the
                # even tiles (sync ring) and odd tiles (scalar ring) each
                # complete in FIFO order within their ring, so each chain
                # only ever waits on its own ring and inter-ring skew
                # cannot stall the reduction
                accE, accO = xt[0], xt[1]
                for l in range(2, L, 2):
                    nc.vector.tensor_add(
                        out=accE[:], in0=accE[:], in1=xt[l][:]
                    )
                    nc.vector.tensor_add(
                        out=accO[:], in0=accO[:], in1=xt[l + 1][:]
                    )
                nc.vector.tensor_add(out=accE[:], in0=accE[:], in1=accO[:])
                return accE

            def project(acc):
                # transpose the block sum into PSUM (single-op groups),
                # one wide PSUM->SBUF copy on ACT, then the K-projection
                pt_all = pt_pool.tile([P, D], F32)
                for dc in range(NDC):
                    nc.tensor.transpose(
                        pt_all[:, dc * P:(dc + 1) * P],
                        acc[:, dc * P:(dc + 1) * P],
                        ident[:],
                    )
                xt_all = tpool.tile([P, D], F32)
                nc.scalar.copy(out=xt_all[:], in_=pt_all[:])

                po = po_pool.tile([P, K], F32)
                # bias broadcast across partitions: ones[1,128].T @ bias[1,64]
                nc.tensor.matmul(
                    po[:], lhsT=ones_sb[:], rhs=bias_sb[:], start=True, stop=False
                )
                for dc in range(NDC):
                    nc.tensor.matmul(
                        po[:],
                        lhsT=xt_all[:, dc * P:(dc + 1) * P],
                        rhs=wt_sb[:, dc * K:(dc + 1) * K],
                        start=False,
                        stop=(dc == NDC - 1),
                    )
                return po

            def finish(blk, po):
                b0 = blk * P
                ot = opool.tile([P, K], F32)
                nc.scalar.activation(
                    ot[:], po[:], mybir.ActivationFunctionType.Tanh
                )
                nc.sync.dma_start(out=yap[b0:b0 + P, :], in_=ot[:])

            # Emission order per block: adds(n) -> loads(n+1) -> psum/matmul
            # stage(n) -> tanh+y(n-1). This keeps every ACT/sync DMA trigger
            # for block n+1 AHEAD of block n's copy/tanh/y in the engine
            # FIFOs, so the two x-stream rings never stall behind compute.
            xt = issue_loads(0)
            prev_po = None
            for blk in range(NBLK):
                acc = reduce(xt)
                if blk + 1 < NBLK:
                    xt = issue_loads(blk + 1)
                po = project(acc)
                if prev_po is not None:
                    finish(blk - 1, prev_po)
                prev_po = po
            finish(NBLK - 1, prev_po)

    nc.compile()
    _nc_cache = nc
    return nc


def _ensure_ntff_hook():
    """Register the axon NTFF profile hook if the image's antenv lacks it."""
    import sys
    import types

    try:
        from antenv.axon_hooks import get_axon_ntff_profile_hook  # noqa: F401
        return
    except ImportError:
        pass
    import antenv

    mod = types.ModuleType("antenv.axon_hooks")
    mod._hook = None

    def set_axon_ntff_profile_hook(h):
        mod._hook = h

    def get_axon_ntff_profile_hook():
        return mod._hook

    mod.set_axon_ntff_profile_hook = set_axon_ntff_profile_hook
    mod.get_axon_ntff_profile_hook = get_axon_ntff_profile_hook
    sys.modules["antenv.axon_hooks"] = mod
    antenv.axon_hooks = mod
    try:
        from trn_agent_boot.trn_boot import _ntff_profile_via_ctypes

        mod._hook = _ntff_profile_via_ctypes("/opt/axon/libaxon_pjrt.so")
    except Exception:
        mod._hook = None


def _run(inputs, trace=False, **kwargs):
    x = np.asarray(inputs["x"], dtype=np.float32)
    W = np.asarray(inputs["W"], dtype=np.float32)
    b = np.asarray(inputs["b"], dtype=np.float32)
    wt = np.ascontiguousarray(W.T).astype(np.float32) * np.float32(1.0 / L)
    bias = np.ascontiguousarray(b.reshape(1, K)).astype(np.float32)
    in_maps = [
        {
            "x": np.ascontiguousarray(x[:, c * BS:(c + 1) * BS, :]),
            "wt": wt,
            "bias": bias,
        }
        for c in range(NCORES)
    ]
    if trace:
        _ensure_ntff_hook()
        import concourse.bass_utils as bu

        bu.upload_artifacts = lambda tmpdir: "local://skipped"
    nc = _build()
    res = run_bass_kernel_spmd(
        nc, in_maps, core_ids=list(range(NCORES)), trace=trace, **kwargs
    )
    y = np.concatenate([r["y"] for r in res.results], axis=0)
    return y, res


def kernel(**inputs):
    y, _ = _run(inputs)
    return y
